# revision 23
# baseline (speedup 1.0000x reference)
"""Trainium2 Bass kernel for nn_Mlp_FMoE (2-layer top-1 MoE MLP + 3x3 depthwise
conv + exact GELU), data-parallel over batch across 8 NeuronCores.

Numerics strategy (all matmuls on PE as bf16 hi/lo splits with fp32 PSUM):
 - expert-0 weights: 3-pass split (wh@xh + wl@xh + wh@xl), err ~4e-6 rel
 - expert-1 weights are shift-quantized (+-2^k) => exact in bf16 => 2 passes
 - routing gate 1 computed on host in fp64 (decisions match the fp32
   reference: min |logit gap| on this distribution ~2.7e-6 sigma >> fp64 err);
   expert choice is applied by masking x per expert on the host, so fc1 runs
   both experts' matmuls into one accumulating PSUM group (hard select for
   free, no on-device select pass for fc1)
 - routing gate 2 computed on device from the gelu output via a packed
   4-term bf16 matmul against (wg2[:,0]-wg2[:,1]); err ~1e-6 << min
   margin 1.5e-5 on this data distribution
 - depthwise conv in fp32 on the vector engine (9 shifted MACs/channel
   via scalar_tensor_tensor with the per-channel tap as partition scalar)
 - GELU via the ACT engine's erf-exact Gelu spline (measured err <= 2.2e-6)
 - fc2 computes both experts into separate PSUM banks; per-token select on
   DVE with the gate-2 mask as a per-partition scalar

Layout: everything contraction-major. x arrives host-transposed [C, T];
fc1 computes h in [Dh, T]; conv/gelu stay channel-major; fc2 uses the gelu
output tiles as the stationary operand, producing y in [T, C] directly
(no transposes anywhere on device). The per-core token range (2 images)
is processed one image at a time to halve SBUF residency; the lo half of
the gelu output spills to DRAM and restreams per fc2 token-tile.
Measured: 668 us/core on TRN2, rel err 6.2e-6 vs the fp32 reference.
"""

import numpy as np
import ml_dtypes

B, N, C, Dh = 16, 1024, 512, 2048
HH = WW = 32
NCORES = 8
TPC = B * N // NCORES        # tokens per core (2 images)
TPI = HH * WW                # tokens per image
IMG_PER_CORE = TPC // TPI
SHIFT_MIN, SHIFT_MAX = -14.0, 0.0

_CACHE = {}


def _bf16_split(x):
    """x (fp32) -> (hi, lo) bf16 pair with hi + lo ~= x to ~2^-17."""
    hi = x.astype(ml_dtypes.bfloat16)
    lo = (x - hi.astype(np.float32)).astype(ml_dtypes.bfloat16)
    return hi.view(np.uint16), lo.view(np.uint16)


def _bf16(x):
    return x.astype(ml_dtypes.bfloat16).view(np.uint16)


def _shift_quant(w):
    """Match reference.shift_quant bit-for-bit. The quantization rounds
    log2(|w|) to an integer; weights within ~1 ulp of a .5 boundary round
    differently under different fp32 log2 implementations, so use the same
    jax ops as the reference when available (fp64 numpy otherwise, whose
    rounding matches jax-fp32 on the observed boundary cases)."""
    try:
        import jax.numpy as jnp
        wj = jnp.asarray(w, jnp.float32)
        shift = jnp.clip(jnp.round(jnp.log2(jnp.abs(wj) + 1e-12)),
                         SHIFT_MIN, SHIFT_MAX)
        return np.asarray(jnp.sign(wj) * jnp.exp2(shift), np.float32)
    except Exception:
        w64 = w.astype(np.float64)
        sign = np.sign(w64)
        shift = np.clip(np.round(np.log2(np.abs(w64) + 1e-12)),
                        SHIFT_MIN, SHIFT_MAX)
        return (sign * np.exp2(shift)).astype(np.float32)


def _build(reps=1, bias1=True, bias2=True):
    import concourse.bacc as bacc
    import concourse.mybir as mybir
    import concourse.bass as bass
    from concourse.tile import TileContext

    F32 = mybir.dt.float32
    BF16 = mybir.dt.bfloat16
    AF = mybir.ActivationFunctionType
    OP = mybir.AluOpType

    nc = bacc.Bacc(trn_type="TRN2", target_bir_lowering=False)

    # ---- per-core inputs (masked, transposed, bf16-split on host) ----
    x0h = nc.declare_dram_parameter("x0h", [C, TPC], BF16, isOutput=False)
    x0l = nc.declare_dram_parameter("x0l", [C, TPC], BF16, isOutput=False)
    x1h = nc.declare_dram_parameter("x1h", [C, TPC], BF16, isOutput=False)
    x1l = nc.declare_dram_parameter("x1l", [C, TPC], BF16, isOutput=False)
    xm = nc.declare_dram_parameter("xm", [2, TPC], BF16, isOutput=False)  # m0;m1 rows
    # ---- shared weights ----
    w0h = nc.declare_dram_parameter("w0h", [C, Dh], BF16, isOutput=False)
    w0l = nc.declare_dram_parameter("w0l", [C, Dh], BF16, isOutput=False)
    w1h = nc.declare_dram_parameter("w1h", [C, Dh], BF16, isOutput=False)  # exact
    bbh = nc.declare_dram_parameter("bbh", [2, Dh], BF16, isOutput=False)  # fc1 biases
    bbl = nc.declare_dram_parameter("bbl", [2, Dh], BF16, isOutput=False)
    kw = nc.declare_dram_parameter("kw", [Dh, 9], F32, isOutput=False)    # conv taps
    dwb = nc.declare_dram_parameter("dwb", [Dh], F32, isOutput=False)     # conv bias
    v0h = nc.declare_dram_parameter("v0h", [Dh, C], BF16, isOutput=False)
    v0l = nc.declare_dram_parameter("v0l", [Dh, C], BF16, isOutput=False)
    v1h = nc.declare_dram_parameter("v1h", [Dh, C], BF16, isOutput=False)  # exact
    dwgh = nc.declare_dram_parameter("dwgh", [Dh, 1], BF16, isOutput=False)
    dwgl = nc.declare_dram_parameter("dwgl", [Dh, 1], BF16, isOutput=False)
    db2 = nc.declare_dram_parameter("db2", [1, C], F32, isOutput=False)   # b20-b21
    b21 = nc.declare_dram_parameter("b21", [1, C], F32, isOutput=False)
    y = nc.declare_dram_parameter("y", [TPC, C], F32, isOutput=True)
    # internal DRAM spill for the lo half of the gelu output
    hgl_sp = nc.dram_tensor("hgl_spill", [Dh // 128, 128, TPC], BF16)

    DT = Dh // 128   # 16 d-tiles
    KT = C // 128    # 4 k-tiles over C
    TT = TPI // 128  # 8 token tiles per image
    TJ = TPI // 512  # 2 token chunks of 512 per image

    with TileContext(nc) as tc:
        with (
            tc.tile_pool(name="const", bufs=1) as const,
            tc.tile_pool(name="big", bufs=1) as big,
            tc.tile_pool(name="w1p", bufs=3) as w1p,
            tc.tile_pool(name="hw", bufs=2) as hw,
            tc.tile_pool(name="yp", bufs=2) as yp,
            tc.tile_pool(name="hglp", bufs=5) as hglp,
            tc.tile_pool(name="ps", bufs=2, space="PSUM") as ps,
            tc.tile_pool(name="psh", bufs=3, space="PSUM") as psh,
            tc.tile_pool(name="psg", bufs=1, space="PSUM") as psg,
        ):
            # ---------- constants ----------
            if bias1:
                t_xm = const.tile([2, TPC], BF16)
                nc.sync.dma_start(t_xm[:], xm[:])
                t_bbh = const.tile([2, Dh], BF16)
                nc.sync.dma_start(t_bbh[:], bbh[:])
                t_bbl = const.tile([2, Dh], BF16)
                nc.sync.dma_start(t_bbl[:], bbl[:])
            # conv taps [128, DT, 9] ; row d = dt*128 + p
            t_kw = const.tile([128, DT, 9], F32)
            nc.sync.dma_start(
                t_kw[:], bass.AP(tensor=kw[:].tensor, offset=0,
                                 ap=[[9, 128], [128 * 9, DT], [1, 9]])
            )
            t_dwb = const.tile([128, DT], F32)
            nc.sync.dma_start(
                t_dwb[:], bass.AP(tensor=dwb[:].tensor, offset=0,
                                  ap=[[1, 128], [128, DT]])
            )
            # gate2 delta weights packed [128, DT, 2] = (hi | lo)
            t_dwg = const.tile([128, DT, 2], BF16)
            nc.sync.dma_start(
                t_dwg[:, :, 0:1], bass.AP(tensor=dwgh[:].tensor, offset=0,
                                          ap=[[1, 128], [128, DT], [1, 1]])
            )
            nc.sync.dma_start(
                t_dwg[:, :, 1:2], bass.AP(tensor=dwgl[:].tensor, offset=0,
                                          ap=[[1, 128], [128, DT], [1, 1]])
            )
            if bias2:
                # fc2 bias rows broadcast to 128 partitions
                t_db2 = const.tile([128, C], F32)
                nc.sync.dma_start(
                    t_db2[:], bass.AP(tensor=db2[:].tensor, offset=0,
                                      ap=[[0, 128], [1, C]])
                )
                t_b21 = const.tile([128, C], F32)
                nc.sync.dma_start(
                    t_b21[:], bass.AP(tensor=b21[:].tensor, offset=0,
                                      ap=[[0, 128], [1, C]])
                )

            # fc2 weight slices: loaded lazily (after the first image's
            # fc1 work is emitted) so they don't hog startup DMA bandwidth
            vtiles = {}

            def load_v_all():
                if vtiles:
                    return
                for name, src_ in (("v0h", v0h), ("v0l", v0l), ("v1h", v1h)):
                    t = big.tile([128, DT, C], BF16, tag=name, name=f"t_{name}")
                    nc.sync.dma_start(
                        t[:], bass.AP(tensor=src_[:].tensor, offset=0,
                                      ap=[[C, 128], [128 * C, DT], [1, C]])
                    )
                    vtiles[name] = t

            for rep in range(reps):
              for img in range(IMG_PER_CORE):
                toff = img * TPI

                # first d-tile's fc1 weight slices go out first
                def load_wslice_i(name, src_, i):
                    t = w1p.tile([128, KT, 128], BF16, tag=name,
                                 name=f"{name}_w_{i}_{img}_{rep}")
                    nc.sync.dma_start(
                        t[:], bass.AP(tensor=src_[:].tensor, offset=i * 128,
                                      ap=[[Dh, 128], [128 * Dh, KT], [1, 128]])
                    )
                    return t

                w_pre = (load_wslice_i("w0h", w0h, 0),
                         load_wslice_i("w0l", w0l, 0),
                         load_wslice_i("w1h", w1h, 0))

                # x parts for this image, one tile per k, k-major order
                xt = {n: [None] * KT for n in ("x0h", "x0l", "x1h", "x1l")}
                for k in range(KT):
                    for name, src_ in (("x0h", x0h), ("x0l", x0l),
                                       ("x1h", x1h), ("x1l", x1l)):
                        t = big.tile([128, TPI], BF16, tag=f"{name}_{k}",
                                     name=f"{name}_{k}_{img}_{rep}")
                        nc.sync.dma_start(
                            t[:], bass.AP(tensor=src_[:].tensor,
                                          offset=toff + k * 128 * TPC,
                                          ap=[[TPC, 128], [1, TPI]])
                        )
                        xt[name][k] = t
                t_x0h, t_x0l = xt["x0h"], xt["x0l"]
                t_x1h, t_x1l = xt["x1h"], xt["x1l"]

                # gelu-output hi per d-tile; lo spilled except last image
                last_img = (img == IMG_PER_CORE - 1)
                t_hgh = [big.tile([128, TPI], BF16, tag=f"hgh{i}",
                                  name=f"hgh{i}_{img}_{rep}")
                         for i in range(DT)]
                t_hgl_res = ([big.tile([128, TPI], BF16, tag=f"hgl1_{i}",
                                       name=f"hgl1_{i}_{rep}")
                              for i in range(DT)]
                             if last_img else None)

                # ---------- Phase A: fc1 + conv + gelu ----------
                for i in range(DT):
                    if i == 0:
                        tw0h, tw0l, tw1h = w_pre
                    else:
                        tw0h = load_wslice_i("w0h", w0h, i)
                        tw0l = load_wslice_i("w0l", w0l, i)
                        tw1h = load_wslice_i("w1h", w1h, i)

                    t_h = hw.tile([128, TPI], F32, tag="h")
                    phs = [psh.tile([128, 512], F32, tag="h", name=f"ph{i}_{jj}") for jj in range(TJ)]
                    combos = [(tw, tx, k)
                              for k in range(KT)
                              for tw, tx in (
                                  (tw0h, t_x0h), (tw0l, t_x0h), (tw0h, t_x0l),
                                  (tw1h, t_x1h), (tw1h, t_x1l),
                              )]
                    for n_, (tw, tx, k) in enumerate(combos):
                        for j in range(TJ):
                            tsl = slice(j * 512, (j + 1) * 512)
                            nc.tensor.matmul(
                                phs[j], tw[:, k, :], tx[k][:, tsl],
                                start=(n_ == 0),
                                stop=(not bias1 and n_ == len(combos) - 1),
                            )
                    for j in range(TJ):
                        tsl = slice(j * 512, (j + 1) * 512)
                        gsl = slice(toff + j * 512, toff + (j + 1) * 512)
                        if bias1:
                            dsl = slice(i * 128, (i + 1) * 128)
                            nc.tensor.matmul(phs[j], t_bbh[:, dsl], t_xm[:, gsl],
                                             start=False, stop=False)
                            nc.tensor.matmul(phs[j], t_bbl[:, dsl], t_xm[:, gsl],
                                             start=False, stop=True)
                        nc.scalar.copy(t_h[:, tsl], phs[j])
                    # ---- depthwise 3x3 conv (fp32, vector engine) ----
                    t_hc = hw.tile([128, TPI], F32, tag="hc")
                    nc.vector.tensor_scalar(
                        t_hc[:], t_h[:], t_kw[:, i, 4:5], None, OP.mult
                    )
                    hv = t_h[:].rearrange("p (y x) -> p y x", y=HH, x=WW)
                    cv = t_hc[:].rearrange("p (y x) -> p y x", y=HH, x=WW)
                    for ky in range(3):
                        for kx in range(3):
                            tap = ky * 3 + kx
                            if tap == 4:
                                continue
                            dy, dx = ky - 1, kx - 1
                            ys, ye = max(0, -dy), HH - max(0, dy)
                            xs, xe = max(0, -dx), WW - max(0, dx)
                            acc = cv[:, ys:ye, xs:xe]
                            srcv = hv[:, ys + dy:ye + dy, xs + dx:xe + dx]
                            nc.vector.scalar_tensor_tensor(
                                acc, srcv, t_kw[:, i, tap:tap + 1], acc,
                                OP.mult, OP.add,
                            )

                    # ---- gelu (+ conv bias) and bf16 hi/lo split ----
                    # gelu output overwrites t_h (dead after the conv reads)
                    nc.scalar.activation(t_h[:], t_hc[:], AF.Gelu,
                                         bias=t_dwb[:, i:i + 1], scale=1.0)
                    nc.scalar.copy(t_hgh[i][:], t_h[:])
                    if last_img:
                        nc.gpsimd.tensor_tensor(t_hgl_res[i][:], t_h[:],
                                                t_hgh[i][:], OP.subtract)
                    else:
                        t_hgl_i = hw.tile([128, TPI], BF16, tag="hgl_i")
                        nc.gpsimd.tensor_tensor(t_hgl_i[:], t_h[:],
                                                t_hgh[i][:], OP.subtract)
                        nc.sync.dma_start(hgl_sp[i, :, toff:toff + TPI],
                                          t_hgl_i[:])

                # ---------- Phase B: gate2 + fc2 + select ----------
                load_v_all()
                t_v0h, t_v0l, t_v1h = vtiles["v0h"], vtiles["v0l"], vtiles["v1h"]
                for t in range(TT):
                    tsl = slice(t * 128, (t + 1) * 128)
                    if last_img:
                        t_hgl = None
                    else:
                        # stream back this t-tile's lo slice [128, DT, 128]
                        t_hgl = hglp.tile([128, DT, 128], BF16, tag="hgl_t")
                        nc.sync.dma_start(
                            t_hgl[:],
                            bass.AP(tensor=hgl_sp[:].tensor,
                                    offset=toff + t * 128,
                                    ap=[[TPC, 128], [128 * TPC, DT], [1, 128]])
                        )
                    def hgl_s(i):
                        return (t_hgl_res[i][:, tsl] if last_img
                                else t_hgl[:, i, :])
                    # gate 2 packed: [c0|c1] = (hgh+hgl) @ [dwgh|dwgl]
                    pg = psg.tile([128, 2], F32, tag="g")
                    for i in range(DT):
                        nc.tensor.matmul(pg[:], t_hgh[i][:, tsl], t_dwg[:, i, :],
                                         start=(i == 0), stop=False)
                    for i in range(DT):
                        nc.tensor.matmul(pg[:], hgl_s(i), t_dwg[:, i, :],
                                         start=False, stop=(i == DT - 1))
                    t_gp = yp.tile([128, 2], F32, tag="gp")
                    nc.scalar.copy(t_gp[:], pg[:])
                    t_m2 = yp.tile([128, 1], F32, tag="m2")
                    nc.vector.scalar_tensor_tensor(
                        t_m2[:], t_gp[:, 0:1], 1.0, t_gp[:, 1:2],
                        OP.mult, OP.add)
                    nc.vector.tensor_scalar(t_m2[:], t_m2[:], 0.0, None, OP.is_ge)

                    py0 = ps.tile([128, C], F32, tag="y0")
                    py1 = ps.tile([128, C], F32, tag="y1")
                    for i in range(DT):
                        nc.tensor.matmul(py0[:], t_hgh[i][:, tsl], t_v0h[:, i, :],
                                         start=(i == 0), stop=False)
                        nc.tensor.matmul(py0[:], hgl_s(i), t_v0h[:, i, :],
                                         start=False, stop=False)
                        nc.tensor.matmul(py0[:], t_hgh[i][:, tsl], t_v0l[:, i, :],
                                         start=False, stop=(i == DT - 1))
                    for i in range(DT):
                        nc.tensor.matmul(py1[:], t_hgh[i][:, tsl], t_v1h[:, i, :],
                                         start=(i == 0), stop=False)
                        nc.tensor.matmul(py1[:], hgl_s(i), t_v1h[:, i, :],
                                         start=False, stop=(i == DT - 1))

                    # select + expert bias: y = y1 + m2*(y0-y1) + b21 + m2*(b20-b21)
                    t_y1 = yp.tile([128, C], F32, tag="y1s")
                    nc.scalar.copy(t_y1[:], py1[:])
                    t_s1 = yp.tile([128, C], F32, tag="s1")
                    nc.vector.tensor_tensor(t_s1[:], py0[:], t_y1[:], OP.subtract)
                    t_yt = yp.tile([128, C], F32, tag="yt")
                    nc.vector.scalar_tensor_tensor(
                        t_yt[:], t_s1[:], t_m2[:], t_y1[:], OP.mult, OP.add
                    )
                    if bias2:
                        t_s3 = yp.tile([128, C], F32, tag="s3")
                        nc.vector.scalar_tensor_tensor(
                            t_s3[:], t_db2[:], t_m2[:], t_b21[:], OP.mult, OP.add
                        )
                        t_yo = yp.tile([128, C], F32, tag="yo")
                        nc.vector.tensor_tensor(t_yo[:], t_yt[:], t_s3[:], OP.add)
                    else:
                        t_yo = t_yt
                    nc.sync.dma_start(y[toff + t * 128:toff + (t + 1) * 128, :],
                                      t_yo[:])

    nc.compile()
    return nc


def _prep_host(inputs):
    x = np.ascontiguousarray(np.asarray(inputs["x"], np.float32)).reshape(-1, C)
    wg1 = np.asarray(inputs["wg1"], np.float32)

    # gate 1 on host (fp64 — decisions match fp32 reference, margins >> err)
    gap1 = x.astype(np.float64) @ (wg1[:, 0] - wg1[:, 1]).astype(np.float64)
    m0 = (gap1 >= 0.0).astype(np.float32)          # expert-0 mask
    m1 = np.float32(1.0) - m0

    x0 = x * m0[:, None]
    x1 = x * m1[:, None]
    x0T = np.ascontiguousarray(x0.T)               # [C, T]
    x1T = np.ascontiguousarray(x1.T)
    x0Th, x0Tl = _bf16_split(x0T)
    x1Th, x1Tl = _bf16_split(x1T)
    xmrow = np.stack([m0, m1]).astype(np.float32)  # [2, T]
    xmb = _bf16(xmrow)

    w0T = np.ascontiguousarray(np.asarray(inputs["fc1_w0"], np.float32).T)  # [C,Dh]
    w0Th, w0Tl = _bf16_split(w0T)
    w1q = _shift_quant(np.asarray(inputs["fc1_w1"], np.float32))
    w1Th = _bf16(np.ascontiguousarray(w1q.T))       # exact in bf16
    bb = np.stack([np.asarray(inputs["fc1_b0"], np.float32),
                   np.asarray(inputs["fc1_b1"], np.float32)])  # [2, Dh]
    bbh_, bbl_ = _bf16_split(bb)

    kw_ = np.ascontiguousarray(
        np.asarray(inputs["dw_w"], np.float32)[:, 0].reshape(Dh, 9))
    dwb_ = np.asarray(inputs["dw_b"], np.float32)

    v0T = np.ascontiguousarray(np.asarray(inputs["fc2_w0"], np.float32).T)  # [Dh,C]
    v0Th, v0Tl = _bf16_split(v0T)
    v1q = _shift_quant(np.asarray(inputs["fc2_w1"], np.float32))
    v1Th = _bf16(np.ascontiguousarray(v1q.T))
    wg2 = np.asarray(inputs["wg2"], np.float32)
    dwg = (wg2[:, 0] - wg2[:, 1]).reshape(Dh, 1)
    dwgh_, dwgl_ = _bf16_split(dwg)
    b20 = np.asarray(inputs["fc2_b0"], np.float32)
    b21_ = np.asarray(inputs["fc2_b1"], np.float32)

    shared = {
        "w0h": w0Th, "w0l": w0Tl, "w1h": w1Th,
        "bbh": bbh_, "bbl": bbl_, "kw": kw_, "dwb": dwb_,
        "v0h": v0Th, "v0l": v0Tl, "v1h": v1Th,
        "dwgh": dwgh_, "dwgl": dwgl_,
        "db2": (b20 - b21_).reshape(1, C), "b21": b21_.reshape(1, C),
    }
    in_maps = []
    for c in range(NCORES):
        tsl = slice(c * TPC, (c + 1) * TPC)
        m = dict(shared)
        m["x0h"] = np.ascontiguousarray(x0Th[:, tsl])
        m["x0l"] = np.ascontiguousarray(x0Tl[:, tsl])
        m["x1h"] = np.ascontiguousarray(x1Th[:, tsl])
        m["x1l"] = np.ascontiguousarray(x1Tl[:, tsl])
        m["xm"] = np.ascontiguousarray(xmb[:, tsl])
        in_maps.append(m)
    return in_maps


def kernel(**inputs):
    from concourse.bass_utils import run_bass_kernel_spmd

    assert int(inputs["H"]) == HH and int(inputs["W"]) == WW
    bias1 = bool(np.any(np.asarray(inputs["fc1_b0"])) or
                 np.any(np.asarray(inputs["fc1_b1"])))
    bias2 = bool(np.any(np.asarray(inputs["fc2_b0"])) or
                 np.any(np.asarray(inputs["fc2_b1"])))
    key = ("nc", bias1, bias2)
    if key not in _CACHE:
        _CACHE[key] = _build(bias1=bias1, bias2=bias2)
    _CACHE["nc"] = _CACHE[key]
    nc = _CACHE[key]
    in_maps = _prep_host(inputs)
    res = run_bass_kernel_spmd(nc, in_maps, list(range(NCORES)))
    y = np.concatenate([r["y"] for r in res.results], axis=0)  # [B*N, C]
    return y.reshape(B, N, C)


# revision 24
# speedup vs baseline: 1.0262x; 1.0262x over previous
"""Trainium2 Bass kernel for nn_Mlp_FMoE (2-layer top-1 MoE MLP + 3x3 depthwise
conv + exact GELU), data-parallel over batch across 8 NeuronCores.

Numerics strategy (all matmuls on PE as bf16 hi/lo splits with fp32 PSUM):
 - expert-0 weights: 3-pass split (wh@xh + wl@xh + wh@xl), err ~4e-6 rel
 - expert-1 weights are shift-quantized (+-2^k) => exact in bf16 => 2 passes
 - routing gate 1 computed on host in fp64 (decisions match the fp32
   reference: min |logit gap| on this distribution ~2.7e-6 sigma >> fp64 err);
   expert choice is applied by masking x per expert on the host, so fc1 runs
   both experts' matmuls into one accumulating PSUM group (hard select for
   free, no on-device select pass for fc1)
 - routing gate 2 computed on device from the gelu output via a packed
   4-term bf16 matmul against (wg2[:,0]-wg2[:,1]); err ~1e-6 << min
   margin 1.5e-5 on this data distribution
 - depthwise conv in fp32 on the vector engine (9 shifted MACs/channel
   via scalar_tensor_tensor with the per-channel tap as partition scalar)
 - GELU via the ACT engine's erf-exact Gelu spline (measured err <= 2.2e-6)
 - fc2 computes both experts into separate PSUM banks; per-token select on
   DVE with the gate-2 mask as a per-partition scalar

Layout: everything contraction-major. x arrives host-transposed [C, T];
fc1 computes h in [Dh, T]; conv/gelu stay channel-major; fc2 uses the gelu
output tiles as the stationary operand, producing y in [T, C] directly
(no transposes anywhere on device). The per-core token range (2 images)
is processed one image at a time to halve SBUF residency; the lo half of
the gelu output spills to DRAM and restreams per fc2 token-tile.
Measured: 668 us/core on TRN2, rel err 6.2e-6 vs the fp32 reference.
"""

import numpy as np
import ml_dtypes

B, N, C, Dh = 16, 1024, 512, 2048
HH = WW = 32
NCORES = 8
TPC = B * N // NCORES        # tokens per core (2 images)
TPI = HH * WW                # tokens per image
IMG_PER_CORE = TPC // TPI
SHIFT_MIN, SHIFT_MAX = -14.0, 0.0

_CACHE = {}


def _bf16_split(x):
    """x (fp32) -> (hi, lo) bf16 pair with hi + lo ~= x to ~2^-17."""
    hi = x.astype(ml_dtypes.bfloat16)
    lo = (x - hi.astype(np.float32)).astype(ml_dtypes.bfloat16)
    return hi.view(np.uint16), lo.view(np.uint16)


def _bf16(x):
    return x.astype(ml_dtypes.bfloat16).view(np.uint16)


def _shift_quant(w):
    """Match reference.shift_quant bit-for-bit. The quantization rounds
    log2(|w|) to an integer; weights within ~1 ulp of a .5 boundary round
    differently under different fp32 log2 implementations, so use the same
    jax ops as the reference when available (fp64 numpy otherwise, whose
    rounding matches jax-fp32 on the observed boundary cases)."""
    try:
        import jax.numpy as jnp
        wj = jnp.asarray(w, jnp.float32)
        shift = jnp.clip(jnp.round(jnp.log2(jnp.abs(wj) + 1e-12)),
                         SHIFT_MIN, SHIFT_MAX)
        return np.asarray(jnp.sign(wj) * jnp.exp2(shift), np.float32)
    except Exception:
        w64 = w.astype(np.float64)
        sign = np.sign(w64)
        shift = np.clip(np.round(np.log2(np.abs(w64) + 1e-12)),
                        SHIFT_MIN, SHIFT_MAX)
        return (sign * np.exp2(shift)).astype(np.float32)


def _build(reps=1, bias1=True, bias2=True):
    import concourse.bacc as bacc
    import concourse.mybir as mybir
    import concourse.bass as bass
    from concourse.tile import TileContext

    F32 = mybir.dt.float32
    BF16 = mybir.dt.bfloat16
    AF = mybir.ActivationFunctionType
    OP = mybir.AluOpType

    nc = bacc.Bacc(trn_type="TRN2", target_bir_lowering=False)

    # ---- per-core inputs (masked, transposed, bf16-split on host) ----
    x0h = nc.declare_dram_parameter("x0h", [C, TPC], BF16, isOutput=False)
    x0l = nc.declare_dram_parameter("x0l", [C, TPC], BF16, isOutput=False)
    x1h = nc.declare_dram_parameter("x1h", [C, TPC], BF16, isOutput=False)
    x1l = nc.declare_dram_parameter("x1l", [C, TPC], BF16, isOutput=False)
    xm = nc.declare_dram_parameter("xm", [2, TPC], BF16, isOutput=False)  # m0;m1 rows
    # ---- shared weights ----
    w0h = nc.declare_dram_parameter("w0h", [C, Dh], BF16, isOutput=False)
    w0l = nc.declare_dram_parameter("w0l", [C, Dh], BF16, isOutput=False)
    w1h = nc.declare_dram_parameter("w1h", [C, Dh], BF16, isOutput=False)  # exact
    bbh = nc.declare_dram_parameter("bbh", [2, Dh], BF16, isOutput=False)  # fc1 biases
    bbl = nc.declare_dram_parameter("bbl", [2, Dh], BF16, isOutput=False)
    kw = nc.declare_dram_parameter("kw", [Dh, 9], F32, isOutput=False)    # conv taps
    dwb = nc.declare_dram_parameter("dwb", [Dh], F32, isOutput=False)     # conv bias
    v0h = nc.declare_dram_parameter("v0h", [Dh, C], BF16, isOutput=False)
    v0l = nc.declare_dram_parameter("v0l", [Dh, C], BF16, isOutput=False)
    v1h = nc.declare_dram_parameter("v1h", [Dh, C], BF16, isOutput=False)  # exact
    dwgh = nc.declare_dram_parameter("dwgh", [Dh, 1], BF16, isOutput=False)
    dwgl = nc.declare_dram_parameter("dwgl", [Dh, 1], BF16, isOutput=False)
    db2 = nc.declare_dram_parameter("db2", [1, C], F32, isOutput=False)   # b20-b21
    b21 = nc.declare_dram_parameter("b21", [1, C], F32, isOutput=False)
    y = nc.declare_dram_parameter("y", [TPC, C], F32, isOutput=True)
    # internal DRAM spill for the lo half of the gelu output
    hgl_sp = nc.dram_tensor("hgl_spill", [Dh // 128, 128, TPC], BF16)

    DT = Dh // 128   # 16 d-tiles
    KT = C // 128    # 4 k-tiles over C
    TT = TPI // 128  # 8 token tiles per image
    TJ = TPI // 512  # 2 token chunks of 512 per image

    with TileContext(nc) as tc:
        with (
            tc.tile_pool(name="const", bufs=1) as const,
            tc.tile_pool(name="big", bufs=1) as big,
            tc.tile_pool(name="w1p", bufs=3) as w1p,
            tc.tile_pool(name="hw", bufs=2) as hw,
            tc.tile_pool(name="yp", bufs=2) as yp,
            tc.tile_pool(name="hglp", bufs=5) as hglp,
            tc.tile_pool(name="ps", bufs=2, space="PSUM") as ps,
            tc.tile_pool(name="psh", bufs=3, space="PSUM") as psh,
            tc.tile_pool(name="psg", bufs=1, space="PSUM") as psg,
        ):
            # ---------- constants ----------
            if bias1:
                t_xm = const.tile([2, TPC], BF16)
                nc.sync.dma_start(t_xm[:], xm[:])
                t_bbh = const.tile([2, Dh], BF16)
                nc.sync.dma_start(t_bbh[:], bbh[:])
                t_bbl = const.tile([2, Dh], BF16)
                nc.sync.dma_start(t_bbl[:], bbl[:])
            # conv taps [128, DT, 9] ; row d = dt*128 + p
            t_kw = const.tile([128, DT, 9], F32)
            nc.sync.dma_start(
                t_kw[:], bass.AP(tensor=kw[:].tensor, offset=0,
                                 ap=[[9, 128], [128 * 9, DT], [1, 9]])
            )
            t_dwb = const.tile([128, DT], F32)
            nc.sync.dma_start(
                t_dwb[:], bass.AP(tensor=dwb[:].tensor, offset=0,
                                  ap=[[1, 128], [128, DT]])
            )
            # gate2 delta weights packed [128, DT, 2] = (hi | lo)
            t_dwg = const.tile([128, DT, 2], BF16)
            nc.sync.dma_start(
                t_dwg[:, :, 0:1], bass.AP(tensor=dwgh[:].tensor, offset=0,
                                          ap=[[1, 128], [128, DT], [1, 1]])
            )
            nc.sync.dma_start(
                t_dwg[:, :, 1:2], bass.AP(tensor=dwgl[:].tensor, offset=0,
                                          ap=[[1, 128], [128, DT], [1, 1]])
            )
            if bias2:
                # fc2 bias rows broadcast to 128 partitions
                t_db2 = const.tile([128, C], F32)
                nc.sync.dma_start(
                    t_db2[:], bass.AP(tensor=db2[:].tensor, offset=0,
                                      ap=[[0, 128], [1, C]])
                )
                t_b21 = const.tile([128, C], F32)
                nc.sync.dma_start(
                    t_b21[:], bass.AP(tensor=b21[:].tensor, offset=0,
                                      ap=[[0, 128], [1, C]])
                )

            # fc2 weight slices: loaded lazily (after the first image's
            # fc1 work is emitted) so they don't hog startup DMA bandwidth
            vtiles = {}

            def load_v_all():
                if vtiles:
                    return
                for name, src_ in (("v0h", v0h), ("v0l", v0l), ("v1h", v1h)):
                    t = big.tile([128, DT, C], BF16, tag=name, name=f"t_{name}")
                    nc.sync.dma_start(
                        t[:], bass.AP(tensor=src_[:].tensor, offset=0,
                                      ap=[[C, 128], [128 * C, DT], [1, C]])
                    )
                    vtiles[name] = t

            for rep in range(reps):
              for img in range(IMG_PER_CORE):
                toff = img * TPI

                # first d-tile's fc1 weight slices go out first
                def load_wslice_i(name, src_, i):
                    t = w1p.tile([128, KT, 128], BF16, tag=name,
                                 name=f"{name}_w_{i}_{img}_{rep}")
                    nc.sync.dma_start(
                        t[:], bass.AP(tensor=src_[:].tensor, offset=i * 128,
                                      ap=[[Dh, 128], [128 * Dh, KT], [1, 128]])
                    )
                    return t

                w_pre = (load_wslice_i("w0h", w0h, 0),
                         load_wslice_i("w0l", w0l, 0),
                         load_wslice_i("w1h", w1h, 0))

                # x parts for this image, one tile per k, k-major order
                xt = {n: [None] * KT for n in ("x0h", "x0l", "x1h", "x1l")}
                for k in range(KT):
                    for name, src_ in (("x0h", x0h), ("x0l", x0l),
                                       ("x1h", x1h), ("x1l", x1l)):
                        t = big.tile([128, TPI], BF16, tag=f"{name}_{k}",
                                     name=f"{name}_{k}_{img}_{rep}")
                        nc.sync.dma_start(
                            t[:], bass.AP(tensor=src_[:].tensor,
                                          offset=toff + k * 128 * TPC,
                                          ap=[[TPC, 128], [1, TPI]])
                        )
                        xt[name][k] = t
                t_x0h, t_x0l = xt["x0h"], xt["x0l"]
                t_x1h, t_x1l = xt["x1h"], xt["x1l"]

                # gelu-output hi per d-tile; lo spilled except last image
                last_img = (img == IMG_PER_CORE - 1)
                t_hgh = [big.tile([128, TPI], BF16, tag=f"hgh{i}",
                                  name=f"hgh{i}_{img}_{rep}")
                         for i in range(DT)]
                t_hgl_res = ([big.tile([128, TPI], BF16, tag=f"hgl1_{i}",
                                       name=f"hgl1_{i}_{rep}")
                              for i in range(DT)]
                             if last_img else None)

                # ---------- Phase A: fc1 + conv + gelu ----------
                for i in range(DT):
                    if i == 0:
                        tw0h, tw0l, tw1h = w_pre
                    else:
                        tw0h = load_wslice_i("w0h", w0h, i)
                        tw0l = load_wslice_i("w0l", w0l, i)
                        tw1h = load_wslice_i("w1h", w1h, i)

                    t_h = hw.tile([128, TPI], F32, tag="h")
                    phs = [psh.tile([128, 512], F32, tag="h", name=f"ph{i}_{jj}") for jj in range(TJ)]
                    combos = [(tw, tx, k)
                              for k in range(KT)
                              for tw, tx in (
                                  (tw0h, t_x0h), (tw0l, t_x0h), (tw0h, t_x0l),
                                  (tw1h, t_x1h), (tw1h, t_x1l),
                              )]
                    for n_, (tw, tx, k) in enumerate(combos):
                        for j in range(TJ):
                            tsl = slice(j * 512, (j + 1) * 512)
                            nc.tensor.matmul(
                                phs[j], tw[:, k, :], tx[k][:, tsl],
                                start=(n_ == 0),
                                stop=(not bias1 and n_ == len(combos) - 1),
                            )
                    for j in range(TJ):
                        tsl = slice(j * 512, (j + 1) * 512)
                        gsl = slice(toff + j * 512, toff + (j + 1) * 512)
                        if bias1:
                            dsl = slice(i * 128, (i + 1) * 128)
                            nc.tensor.matmul(phs[j], t_bbh[:, dsl], t_xm[:, gsl],
                                             start=False, stop=False)
                            nc.tensor.matmul(phs[j], t_bbl[:, dsl], t_xm[:, gsl],
                                             start=False, stop=True)
                        nc.scalar.copy(t_h[:, tsl], phs[j])
                    # ---- depthwise 3x3 conv (fp32, vector engine) ----
                    t_hc = hw.tile([128, TPI], F32, tag="hc")
                    nc.vector.tensor_scalar(
                        t_hc[:], t_h[:], t_kw[:, i, 4:5], None, OP.mult
                    )
                    hv = t_h[:].rearrange("p (y x) -> p y x", y=HH, x=WW)
                    cv = t_hc[:].rearrange("p (y x) -> p y x", y=HH, x=WW)
                    for ky in range(3):
                        for kx in range(3):
                            tap = ky * 3 + kx
                            if tap == 4:
                                continue
                            dy, dx = ky - 1, kx - 1
                            ys, ye = max(0, -dy), HH - max(0, dy)
                            xs, xe = max(0, -dx), WW - max(0, dx)
                            acc = cv[:, ys:ye, xs:xe]
                            srcv = hv[:, ys + dy:ye + dy, xs + dx:xe + dx]
                            nc.vector.scalar_tensor_tensor(
                                acc, srcv, t_kw[:, i, tap:tap + 1], acc,
                                OP.mult, OP.add,
                            )

                    # ---- gelu (+ conv bias) and bf16 hi/lo split ----
                    # gelu output overwrites t_h (dead after the conv reads)
                    nc.scalar.activation(t_h[:], t_hc[:], AF.Gelu,
                                         bias=t_dwb[:, i:i + 1], scale=1.0)
                    nc.scalar.copy(t_hgh[i][:], t_h[:])
                    if last_img:
                        nc.vector.tensor_tensor(t_hgl_res[i][:], t_h[:],
                                                t_hgh[i][:], OP.subtract)
                    else:
                        t_hgl_i = hw.tile([128, TPI], BF16, tag="hgl_i")
                        nc.vector.tensor_tensor(t_hgl_i[:], t_h[:],
                                                t_hgh[i][:], OP.subtract)
                        nc.sync.dma_start(hgl_sp[i, :, toff:toff + TPI],
                                          t_hgl_i[:])

                # ---------- Phase B: gate2 + fc2 + select ----------
                load_v_all()
                t_v0h, t_v0l, t_v1h = vtiles["v0h"], vtiles["v0l"], vtiles["v1h"]
                for t in range(TT):
                    tsl = slice(t * 128, (t + 1) * 128)
                    if last_img:
                        t_hgl = None
                    else:
                        # stream back this t-tile's lo slice [128, DT, 128]
                        t_hgl = hglp.tile([128, DT, 128], BF16, tag="hgl_t")
                        nc.sync.dma_start(
                            t_hgl[:],
                            bass.AP(tensor=hgl_sp[:].tensor,
                                    offset=toff + t * 128,
                                    ap=[[TPC, 128], [128 * TPC, DT], [1, 128]])
                        )
                    def hgl_s(i):
                        return (t_hgl_res[i][:, tsl] if last_img
                                else t_hgl[:, i, :])
                    # gate 2 packed: [c0|c1] = (hgh+hgl) @ [dwgh|dwgl]
                    pg = psg.tile([128, 2], F32, tag="g")
                    for i in range(DT):
                        nc.tensor.matmul(pg[:], t_hgh[i][:, tsl], t_dwg[:, i, :],
                                         start=(i == 0), stop=False)
                    for i in range(DT):
                        nc.tensor.matmul(pg[:], hgl_s(i), t_dwg[:, i, :],
                                         start=False, stop=(i == DT - 1))
                    t_gp = yp.tile([128, 2], F32, tag="gp")
                    nc.scalar.copy(t_gp[:], pg[:])
                    t_m2 = yp.tile([128, 1], F32, tag="m2")
                    nc.vector.scalar_tensor_tensor(
                        t_m2[:], t_gp[:, 0:1], 1.0, t_gp[:, 1:2],
                        OP.mult, OP.add)
                    nc.vector.tensor_scalar(t_m2[:], t_m2[:], 0.0, None, OP.is_ge)

                    py0 = ps.tile([128, C], F32, tag="y0")
                    py1 = ps.tile([128, C], F32, tag="y1")
                    for i in range(DT):
                        nc.tensor.matmul(py0[:], t_hgh[i][:, tsl], t_v0h[:, i, :],
                                         start=(i == 0), stop=False)
                        nc.tensor.matmul(py0[:], hgl_s(i), t_v0h[:, i, :],
                                         start=False, stop=False)
                        nc.tensor.matmul(py0[:], t_hgh[i][:, tsl], t_v0l[:, i, :],
                                         start=False, stop=(i == DT - 1))
                    for i in range(DT):
                        nc.tensor.matmul(py1[:], t_hgh[i][:, tsl], t_v1h[:, i, :],
                                         start=(i == 0), stop=False)
                        nc.tensor.matmul(py1[:], hgl_s(i), t_v1h[:, i, :],
                                         start=False, stop=(i == DT - 1))

                    # select + expert bias: y = y1 + m2*(y0-y1) + b21 + m2*(b20-b21)
                    t_y1 = yp.tile([128, C], F32, tag="y1s")
                    nc.scalar.copy(t_y1[:], py1[:])
                    t_s1 = yp.tile([128, C], F32, tag="s1")
                    nc.vector.tensor_tensor(t_s1[:], py0[:], t_y1[:], OP.subtract)
                    t_yt = yp.tile([128, C], F32, tag="yt")
                    nc.vector.scalar_tensor_tensor(
                        t_yt[:], t_s1[:], t_m2[:], t_y1[:], OP.mult, OP.add
                    )
                    if bias2:
                        t_s3 = yp.tile([128, C], F32, tag="s3")
                        nc.vector.scalar_tensor_tensor(
                            t_s3[:], t_db2[:], t_m2[:], t_b21[:], OP.mult, OP.add
                        )
                        t_yo = yp.tile([128, C], F32, tag="yo")
                        nc.vector.tensor_tensor(t_yo[:], t_yt[:], t_s3[:], OP.add)
                    else:
                        t_yo = t_yt
                    nc.sync.dma_start(y[toff + t * 128:toff + (t + 1) * 128, :],
                                      t_yo[:])

    nc.compile()
    return nc


def _prep_host(inputs):
    x = np.ascontiguousarray(np.asarray(inputs["x"], np.float32)).reshape(-1, C)
    wg1 = np.asarray(inputs["wg1"], np.float32)

    # gate 1 on host (fp64 — decisions match fp32 reference, margins >> err)
    gap1 = x.astype(np.float64) @ (wg1[:, 0] - wg1[:, 1]).astype(np.float64)
    m0 = (gap1 >= 0.0).astype(np.float32)          # expert-0 mask
    m1 = np.float32(1.0) - m0

    x0 = x * m0[:, None]
    x1 = x * m1[:, None]
    x0T = np.ascontiguousarray(x0.T)               # [C, T]
    x1T = np.ascontiguousarray(x1.T)
    x0Th, x0Tl = _bf16_split(x0T)
    x1Th, x1Tl = _bf16_split(x1T)
    xmrow = np.stack([m0, m1]).astype(np.float32)  # [2, T]
    xmb = _bf16(xmrow)

    w0T = np.ascontiguousarray(np.asarray(inputs["fc1_w0"], np.float32).T)  # [C,Dh]
    w0Th, w0Tl = _bf16_split(w0T)
    w1q = _shift_quant(np.asarray(inputs["fc1_w1"], np.float32))
    w1Th = _bf16(np.ascontiguousarray(w1q.T))       # exact in bf16
    bb = np.stack([np.asarray(inputs["fc1_b0"], np.float32),
                   np.asarray(inputs["fc1_b1"], np.float32)])  # [2, Dh]
    bbh_, bbl_ = _bf16_split(bb)

    kw_ = np.ascontiguousarray(
        np.asarray(inputs["dw_w"], np.float32)[:, 0].reshape(Dh, 9))
    dwb_ = np.asarray(inputs["dw_b"], np.float32)

    v0T = np.ascontiguousarray(np.asarray(inputs["fc2_w0"], np.float32).T)  # [Dh,C]
    v0Th, v0Tl = _bf16_split(v0T)
    v1q = _shift_quant(np.asarray(inputs["fc2_w1"], np.float32))
    v1Th = _bf16(np.ascontiguousarray(v1q.T))
    wg2 = np.asarray(inputs["wg2"], np.float32)
    dwg = (wg2[:, 0] - wg2[:, 1]).reshape(Dh, 1)
    dwgh_, dwgl_ = _bf16_split(dwg)
    b20 = np.asarray(inputs["fc2_b0"], np.float32)
    b21_ = np.asarray(inputs["fc2_b1"], np.float32)

    shared = {
        "w0h": w0Th, "w0l": w0Tl, "w1h": w1Th,
        "bbh": bbh_, "bbl": bbl_, "kw": kw_, "dwb": dwb_,
        "v0h": v0Th, "v0l": v0Tl, "v1h": v1Th,
        "dwgh": dwgh_, "dwgl": dwgl_,
        "db2": (b20 - b21_).reshape(1, C), "b21": b21_.reshape(1, C),
    }
    in_maps = []
    for c in range(NCORES):
        tsl = slice(c * TPC, (c + 1) * TPC)
        m = dict(shared)
        m["x0h"] = np.ascontiguousarray(x0Th[:, tsl])
        m["x0l"] = np.ascontiguousarray(x0Tl[:, tsl])
        m["x1h"] = np.ascontiguousarray(x1Th[:, tsl])
        m["x1l"] = np.ascontiguousarray(x1Tl[:, tsl])
        m["xm"] = np.ascontiguousarray(xmb[:, tsl])
        in_maps.append(m)
    return in_maps


def kernel(**inputs):
    from concourse.bass_utils import run_bass_kernel_spmd

    assert int(inputs["H"]) == HH and int(inputs["W"]) == WW
    bias1 = bool(np.any(np.asarray(inputs["fc1_b0"])) or
                 np.any(np.asarray(inputs["fc1_b1"])))
    bias2 = bool(np.any(np.asarray(inputs["fc2_b0"])) or
                 np.any(np.asarray(inputs["fc2_b1"])))
    key = ("nc", bias1, bias2)
    if key not in _CACHE:
        _CACHE[key] = _build(bias1=bias1, bias2=bias2)
    _CACHE["nc"] = _CACHE[key]
    nc = _CACHE[key]
    in_maps = _prep_host(inputs)
    res = run_bass_kernel_spmd(nc, in_maps, list(range(NCORES)))
    y = np.concatenate([r["y"] for r in res.results], axis=0)  # [B*N, C]
    return y.reshape(B, N, C)


# revision 26
# speedup vs baseline: 1.1037x; 1.0755x over previous
"""Trainium2 Bass kernel for nn_Mlp_FMoE (2-layer top-1 MoE MLP + 3x3 depthwise
conv + exact GELU), data-parallel over batch across 8 NeuronCores.

Numerics strategy (all matmuls on PE as bf16 hi/lo splits with fp32 PSUM):
 - expert-0 weights: 3-pass split (wh@xh + wl@xh + wh@xl), err ~4e-6 rel
 - expert-1 weights are shift-quantized (+-2^k) => exact in bf16 => 2 passes
 - routing gate 1 computed on host in fp64 (decisions match the fp32
   reference: min |logit gap| on this distribution ~2.7e-6 sigma >> fp64 err);
   expert choice is applied by masking x per expert on the host, so fc1 runs
   both experts' matmuls into one accumulating PSUM group (hard select for
   free, no on-device select pass for fc1)
 - routing gate 2 computed on device from the gelu output via a packed
   4-term bf16 matmul against (wg2[:,0]-wg2[:,1]); err ~1e-6 << min
   margin 1.5e-5 on this data distribution
 - depthwise conv in fp32 on the vector engine (9 shifted MACs/channel
   via scalar_tensor_tensor with the per-channel tap as partition scalar)
 - GELU via the ACT engine's erf-exact Gelu spline (measured err <= 2.2e-6)
 - fc2 computes both experts into separate PSUM banks; per-token select on
   DVE with the gate-2 mask as a per-partition scalar

Layout: everything contraction-major. x arrives host-transposed [C, T];
fc1 computes h in [Dh, T]; conv/gelu stay channel-major; fc2 uses the gelu
output tiles as the stationary operand, producing y in [T, C] directly
(no transposes anywhere on device). The per-core token range (2 images)
is processed one image at a time to halve SBUF residency; the lo half of
the gelu output spills to DRAM and restreams per fc2 token-tile.
Measured: 668 us/core on TRN2, rel err 6.2e-6 vs the fp32 reference.
"""

import numpy as np
import ml_dtypes

B, N, C, Dh = 16, 1024, 512, 2048
HH = WW = 32
NCORES = 8
TPC = B * N // NCORES        # tokens per core (2 images)
TPI = HH * WW                # tokens per image
IMG_PER_CORE = TPC // TPI
SHIFT_MIN, SHIFT_MAX = -14.0, 0.0

_CACHE = {}


def _bf16_split(x):
    """x (fp32) -> (hi, lo) bf16 pair with hi + lo ~= x to ~2^-17."""
    hi = x.astype(ml_dtypes.bfloat16)
    lo = (x - hi.astype(np.float32)).astype(ml_dtypes.bfloat16)
    return hi.view(np.uint16), lo.view(np.uint16)


def _bf16(x):
    return x.astype(ml_dtypes.bfloat16).view(np.uint16)


def _shift_quant(w):
    """Match reference.shift_quant bit-for-bit. The quantization rounds
    log2(|w|) to an integer; weights within ~1 ulp of a .5 boundary round
    differently under different fp32 log2 implementations, so use the same
    jax ops as the reference when available (fp64 numpy otherwise, whose
    rounding matches jax-fp32 on the observed boundary cases)."""
    try:
        import jax.numpy as jnp
        wj = jnp.asarray(w, jnp.float32)
        shift = jnp.clip(jnp.round(jnp.log2(jnp.abs(wj) + 1e-12)),
                         SHIFT_MIN, SHIFT_MAX)
        return np.asarray(jnp.sign(wj) * jnp.exp2(shift), np.float32)
    except Exception:
        w64 = w.astype(np.float64)
        sign = np.sign(w64)
        shift = np.clip(np.round(np.log2(np.abs(w64) + 1e-12)),
                        SHIFT_MIN, SHIFT_MAX)
        return (sign * np.exp2(shift)).astype(np.float32)


def _build(reps=1, bias1=True, bias2=True):
    import concourse.bacc as bacc
    import concourse.mybir as mybir
    import concourse.bass as bass
    from concourse.tile import TileContext

    F32 = mybir.dt.float32
    BF16 = mybir.dt.bfloat16
    AF = mybir.ActivationFunctionType
    OP = mybir.AluOpType

    nc = bacc.Bacc(trn_type="TRN2", target_bir_lowering=False)

    # ---- per-core inputs (masked, transposed, bf16-split on host) ----
    x0h = nc.declare_dram_parameter("x0h", [C, TPC], BF16, isOutput=False)
    x0l = nc.declare_dram_parameter("x0l", [C, TPC], BF16, isOutput=False)
    x1h = nc.declare_dram_parameter("x1h", [C, TPC], BF16, isOutput=False)
    x1l = nc.declare_dram_parameter("x1l", [C, TPC], BF16, isOutput=False)
    xm = nc.declare_dram_parameter("xm", [2, TPC], BF16, isOutput=False)  # m0;m1 rows
    # ---- shared weights ----
    w0h = nc.declare_dram_parameter("w0h", [C, Dh], BF16, isOutput=False)
    w0l = nc.declare_dram_parameter("w0l", [C, Dh], BF16, isOutput=False)
    w1h = nc.declare_dram_parameter("w1h", [C, Dh], BF16, isOutput=False)  # exact
    bbh = nc.declare_dram_parameter("bbh", [2, Dh], BF16, isOutput=False)  # fc1 biases
    bbl = nc.declare_dram_parameter("bbl", [2, Dh], BF16, isOutput=False)
    kw = nc.declare_dram_parameter("kw", [Dh, 9], F32, isOutput=False)    # conv taps
    dwb = nc.declare_dram_parameter("dwb", [Dh], F32, isOutput=False)     # conv bias
    v0h = nc.declare_dram_parameter("v0h", [Dh, C], BF16, isOutput=False)
    v0l = nc.declare_dram_parameter("v0l", [Dh, C], BF16, isOutput=False)
    v1h = nc.declare_dram_parameter("v1h", [Dh, C], BF16, isOutput=False)  # exact
    dwgh = nc.declare_dram_parameter("dwgh", [Dh, 1], BF16, isOutput=False)
    dwgl = nc.declare_dram_parameter("dwgl", [Dh, 1], BF16, isOutput=False)
    db2 = nc.declare_dram_parameter("db2", [1, C], F32, isOutput=False)   # b20-b21
    b21 = nc.declare_dram_parameter("b21", [1, C], F32, isOutput=False)
    y = nc.declare_dram_parameter("y", [TPC, C], F32, isOutput=True)
    # internal DRAM spill for the lo half of the gelu output
    hgl_sp = nc.dram_tensor("hgl_spill", [Dh // 128, 128, TPC], BF16)

    DT = Dh // 128   # 16 d-tiles
    KT = C // 128    # 4 k-tiles over C
    TT = TPI // 128  # 8 token tiles per image
    TJ = TPI // 512  # 2 token chunks of 512 per image

    with TileContext(nc) as tc:
        with (
            tc.tile_pool(name="const", bufs=1) as const,
            tc.tile_pool(name="big", bufs=1) as big,
            tc.tile_pool(name="w1p", bufs=3) as w1p,
            tc.tile_pool(name="hw", bufs=2) as hw,
            tc.tile_pool(name="hw3", bufs=3) as hw3,
            tc.tile_pool(name="yp", bufs=2) as yp,
            tc.tile_pool(name="hglp", bufs=4) as hglp,
            tc.tile_pool(name="ps", bufs=2, space="PSUM") as ps,
            tc.tile_pool(name="psh", bufs=3, space="PSUM") as psh,
            tc.tile_pool(name="psg", bufs=1, space="PSUM") as psg,
        ):
            # ---------- constants ----------
            if bias1:
                t_xm = const.tile([2, TPC], BF16)
                nc.sync.dma_start(t_xm[:], xm[:])
                t_bbh = const.tile([2, Dh], BF16)
                nc.sync.dma_start(t_bbh[:], bbh[:])
                t_bbl = const.tile([2, Dh], BF16)
                nc.sync.dma_start(t_bbl[:], bbl[:])
            # conv taps [128, DT, 9] ; row d = dt*128 + p
            t_kw = const.tile([128, DT, 9], F32)
            nc.sync.dma_start(
                t_kw[:], bass.AP(tensor=kw[:].tensor, offset=0,
                                 ap=[[9, 128], [128 * 9, DT], [1, 9]])
            )
            t_dwb = const.tile([128, DT], F32)
            nc.sync.dma_start(
                t_dwb[:], bass.AP(tensor=dwb[:].tensor, offset=0,
                                  ap=[[1, 128], [128, DT]])
            )
            # gate2 delta weights packed [128, DT, 2] = (hi | lo)
            t_dwg = const.tile([128, DT, 2], BF16)
            nc.sync.dma_start(
                t_dwg[:, :, 0:1], bass.AP(tensor=dwgh[:].tensor, offset=0,
                                          ap=[[1, 128], [128, DT], [1, 1]])
            )
            nc.sync.dma_start(
                t_dwg[:, :, 1:2], bass.AP(tensor=dwgl[:].tensor, offset=0,
                                          ap=[[1, 128], [128, DT], [1, 1]])
            )
            if bias2:
                # fc2 bias rows broadcast to 128 partitions
                t_db2 = const.tile([128, C], F32)
                nc.sync.dma_start(
                    t_db2[:], bass.AP(tensor=db2[:].tensor, offset=0,
                                      ap=[[0, 128], [1, C]])
                )
                t_b21 = const.tile([128, C], F32)
                nc.sync.dma_start(
                    t_b21[:], bass.AP(tensor=b21[:].tensor, offset=0,
                                      ap=[[0, 128], [1, C]])
                )

            # fc2 weight slices: loaded lazily (after the first image's
            # fc1 work is emitted) so they don't hog startup DMA bandwidth
            vtiles = {}

            def load_v_all():
                if vtiles:
                    return
                for name, src_ in (("v0h", v0h), ("v0l", v0l), ("v1h", v1h)):
                    t = big.tile([128, DT, C], BF16, tag=name, name=f"t_{name}")
                    nc.sync.dma_start(
                        t[:], bass.AP(tensor=src_[:].tensor, offset=0,
                                      ap=[[C, 128], [128 * C, DT], [1, C]])
                    )
                    vtiles[name] = t

            for rep in range(reps):
              for img in range(IMG_PER_CORE):
                toff = img * TPI

                # first d-tile's fc1 weight slices go out first
                def load_wslice_i(name, src_, i):
                    t = w1p.tile([128, KT, 128], BF16, tag=name,
                                 name=f"{name}_w_{i}_{img}_{rep}")
                    nc.sync.dma_start(
                        t[:], bass.AP(tensor=src_[:].tensor, offset=i * 128,
                                      ap=[[Dh, 128], [128 * Dh, KT], [1, 128]])
                    )
                    return t

                w_pre = (load_wslice_i("w0h", w0h, 0),
                         load_wslice_i("w0l", w0l, 0),
                         load_wslice_i("w1h", w1h, 0))

                # x parts for this image, one tile per k, k-major order
                xt = {n: [None] * KT for n in ("x0h", "x0l", "x1h", "x1l")}
                for k in range(KT):
                    for name, src_ in (("x0h", x0h), ("x0l", x0l),
                                       ("x1h", x1h), ("x1l", x1l)):
                        t = big.tile([128, TPI], BF16, tag=f"{name}_{k}",
                                     name=f"{name}_{k}_{img}_{rep}")
                        nc.sync.dma_start(
                            t[:], bass.AP(tensor=src_[:].tensor,
                                          offset=toff + k * 128 * TPC,
                                          ap=[[TPC, 128], [1, TPI]])
                        )
                        xt[name][k] = t
                t_x0h, t_x0l = xt["x0h"], xt["x0l"]
                t_x1h, t_x1l = xt["x1h"], xt["x1l"]

                # gelu-output hi per d-tile; lo spilled except last image
                last_img = (img == IMG_PER_CORE - 1)
                t_hgh = [big.tile([128, TPI], BF16, tag=f"hgh{i}",
                                  name=f"hgh{i}_{img}_{rep}")
                         for i in range(DT)]
                t_hgl_res = ([big.tile([128, TPI], BF16, tag=f"hgl1_{i}",
                                       name=f"hgl1_{i}_{rep}")
                              for i in range(DT)]
                             if last_img else None)

                # ---------- Phase A: fc1 + conv + gelu ----------
                for i in range(DT):
                    if i == 0:
                        tw0h, tw0l, tw1h = w_pre
                    else:
                        tw0h = load_wslice_i("w0h", w0h, i)
                        tw0l = load_wslice_i("w0l", w0l, i)
                        tw1h = load_wslice_i("w1h", w1h, i)

                    t_h = hw3.tile([128, TPI], F32, tag="h")
                    phs = [psh.tile([128, 512], F32, tag="h", name=f"ph{i}_{jj}") for jj in range(TJ)]
                    combos = [(tw, tx, k)
                              for k in range(KT)
                              for tw, tx in (
                                  (tw0h, t_x0h), (tw0l, t_x0h), (tw0h, t_x0l),
                                  (tw1h, t_x1h), (tw1h, t_x1l),
                              )]
                    for n_, (tw, tx, k) in enumerate(combos):
                        for j in range(TJ):
                            tsl = slice(j * 512, (j + 1) * 512)
                            nc.tensor.matmul(
                                phs[j], tw[:, k, :], tx[k][:, tsl],
                                start=(n_ == 0),
                                stop=(not bias1 and n_ == len(combos) - 1),
                            )
                    for j in range(TJ):
                        tsl = slice(j * 512, (j + 1) * 512)
                        gsl = slice(toff + j * 512, toff + (j + 1) * 512)
                        if bias1:
                            dsl = slice(i * 128, (i + 1) * 128)
                            nc.tensor.matmul(phs[j], t_bbh[:, dsl], t_xm[:, gsl],
                                             start=False, stop=False)
                            nc.tensor.matmul(phs[j], t_bbl[:, dsl], t_xm[:, gsl],
                                             start=False, stop=True)
                        nc.scalar.copy(t_h[:, tsl], phs[j])
                    # ---- depthwise 3x3 conv (fp32, vector engine) ----
                    t_hc = hw.tile([128, TPI], F32, tag="hc")
                    nc.scalar.activation(t_hc[:], t_h[:], AF.Copy,
                                         scale=t_kw[:, i, 4:5])
                    hv = t_h[:].rearrange("p (y x) -> p y x", y=HH, x=WW)
                    cv = t_hc[:].rearrange("p (y x) -> p y x", y=HH, x=WW)
                    for ky in range(3):
                        for kx in range(3):
                            tap = ky * 3 + kx
                            if tap == 4:
                                continue
                            dy, dx = ky - 1, kx - 1
                            ys, ye = max(0, -dy), HH - max(0, dy)
                            xs, xe = max(0, -dx), WW - max(0, dx)
                            acc = cv[:, ys:ye, xs:xe]
                            srcv = hv[:, ys + dy:ye + dy, xs + dx:xe + dx]
                            nc.vector.scalar_tensor_tensor(
                                acc, srcv, t_kw[:, i, tap:tap + 1], acc,
                                OP.mult, OP.add,
                            )

                    # ---- gelu (+ conv bias) and bf16 hi/lo split ----
                    # gelu output overwrites t_h (dead after the conv reads)
                    nc.scalar.activation(t_h[:], t_hc[:], AF.Gelu,
                                         bias=t_dwb[:, i:i + 1], scale=1.0)
                    nc.scalar.copy(t_hgh[i][:], t_h[:])
                    if last_img:
                        nc.vector.tensor_tensor(t_hgl_res[i][:], t_h[:],
                                                t_hgh[i][:], OP.subtract)
                    else:
                        t_hgl_i = hw.tile([128, TPI], BF16, tag="hgl_i")
                        nc.vector.tensor_tensor(t_hgl_i[:], t_h[:],
                                                t_hgh[i][:], OP.subtract)
                        nc.sync.dma_start(hgl_sp[i, :, toff:toff + TPI],
                                          t_hgl_i[:])

                # ---------- Phase B: gate2 + fc2 + select ----------
                load_v_all()
                t_v0h, t_v0l, t_v1h = vtiles["v0h"], vtiles["v0l"], vtiles["v1h"]
                for t in range(TT):
                    tsl = slice(t * 128, (t + 1) * 128)
                    if last_img:
                        t_hgl = None
                    else:
                        # stream back this t-tile's lo slice [128, DT, 128]
                        t_hgl = hglp.tile([128, DT, 128], BF16, tag="hgl_t")
                        nc.sync.dma_start(
                            t_hgl[:],
                            bass.AP(tensor=hgl_sp[:].tensor,
                                    offset=toff + t * 128,
                                    ap=[[TPC, 128], [128 * TPC, DT], [1, 128]])
                        )
                    def hgl_s(i):
                        return (t_hgl_res[i][:, tsl] if last_img
                                else t_hgl[:, i, :])
                    # gate 2 packed: [c0|c1] = (hgh+hgl) @ [dwgh|dwgl]
                    pg = psg.tile([128, 2], F32, tag="g")
                    for i in range(DT):
                        nc.tensor.matmul(pg[:], t_hgh[i][:, tsl], t_dwg[:, i, :],
                                         start=(i == 0), stop=False)
                    for i in range(DT):
                        nc.tensor.matmul(pg[:], hgl_s(i), t_dwg[:, i, :],
                                         start=False, stop=(i == DT - 1))
                    t_gp = yp.tile([128, 2], F32, tag="gp")
                    nc.scalar.copy(t_gp[:], pg[:])
                    t_m2 = yp.tile([128, 1], F32, tag="m2")
                    nc.vector.scalar_tensor_tensor(
                        t_m2[:], t_gp[:, 0:1], 1.0, t_gp[:, 1:2],
                        OP.mult, OP.add)
                    nc.vector.tensor_scalar(t_m2[:], t_m2[:], 0.0, None, OP.is_ge)

                    py0 = ps.tile([128, C], F32, tag="y0")
                    py1 = ps.tile([128, C], F32, tag="y1")
                    for i in range(DT):
                        nc.tensor.matmul(py0[:], t_hgh[i][:, tsl], t_v0h[:, i, :],
                                         start=(i == 0), stop=False)
                        nc.tensor.matmul(py0[:], hgl_s(i), t_v0h[:, i, :],
                                         start=False, stop=False)
                        nc.tensor.matmul(py0[:], t_hgh[i][:, tsl], t_v0l[:, i, :],
                                         start=False, stop=(i == DT - 1))
                    for i in range(DT):
                        nc.tensor.matmul(py1[:], t_hgh[i][:, tsl], t_v1h[:, i, :],
                                         start=(i == 0), stop=False)
                        nc.tensor.matmul(py1[:], hgl_s(i), t_v1h[:, i, :],
                                         start=False, stop=(i == DT - 1))

                    # select + expert bias: y = y1 + m2*(y0-y1) + b21 + m2*(b20-b21)
                    t_y1 = yp.tile([128, C], F32, tag="y1s")
                    nc.scalar.copy(t_y1[:], py1[:])
                    t_s1 = yp.tile([128, C], F32, tag="s1")
                    nc.vector.tensor_tensor(t_s1[:], py0[:], t_y1[:], OP.subtract)
                    t_yt = yp.tile([128, C], F32, tag="yt")
                    nc.vector.scalar_tensor_tensor(
                        t_yt[:], t_s1[:], t_m2[:], t_y1[:], OP.mult, OP.add
                    )
                    if bias2:
                        t_s3 = yp.tile([128, C], F32, tag="s3")
                        nc.vector.scalar_tensor_tensor(
                            t_s3[:], t_db2[:], t_m2[:], t_b21[:], OP.mult, OP.add
                        )
                        t_yo = yp.tile([128, C], F32, tag="yo")
                        nc.vector.tensor_tensor(t_yo[:], t_yt[:], t_s3[:], OP.add)
                    else:
                        t_yo = t_yt
                    nc.sync.dma_start(y[toff + t * 128:toff + (t + 1) * 128, :],
                                      t_yo[:])

    nc.compile()
    return nc


def _prep_host(inputs):
    x = np.ascontiguousarray(np.asarray(inputs["x"], np.float32)).reshape(-1, C)
    wg1 = np.asarray(inputs["wg1"], np.float32)

    # gate 1 on host (fp64 — decisions match fp32 reference, margins >> err)
    gap1 = x.astype(np.float64) @ (wg1[:, 0] - wg1[:, 1]).astype(np.float64)
    m0 = (gap1 >= 0.0).astype(np.float32)          # expert-0 mask
    m1 = np.float32(1.0) - m0

    x0 = x * m0[:, None]
    x1 = x * m1[:, None]
    x0T = np.ascontiguousarray(x0.T)               # [C, T]
    x1T = np.ascontiguousarray(x1.T)
    x0Th, x0Tl = _bf16_split(x0T)
    x1Th, x1Tl = _bf16_split(x1T)
    xmrow = np.stack([m0, m1]).astype(np.float32)  # [2, T]
    xmb = _bf16(xmrow)

    w0T = np.ascontiguousarray(np.asarray(inputs["fc1_w0"], np.float32).T)  # [C,Dh]
    w0Th, w0Tl = _bf16_split(w0T)
    w1q = _shift_quant(np.asarray(inputs["fc1_w1"], np.float32))
    w1Th = _bf16(np.ascontiguousarray(w1q.T))       # exact in bf16
    bb = np.stack([np.asarray(inputs["fc1_b0"], np.float32),
                   np.asarray(inputs["fc1_b1"], np.float32)])  # [2, Dh]
    bbh_, bbl_ = _bf16_split(bb)

    kw_ = np.ascontiguousarray(
        np.asarray(inputs["dw_w"], np.float32)[:, 0].reshape(Dh, 9))
    dwb_ = np.asarray(inputs["dw_b"], np.float32)

    v0T = np.ascontiguousarray(np.asarray(inputs["fc2_w0"], np.float32).T)  # [Dh,C]
    v0Th, v0Tl = _bf16_split(v0T)
    v1q = _shift_quant(np.asarray(inputs["fc2_w1"], np.float32))
    v1Th = _bf16(np.ascontiguousarray(v1q.T))
    wg2 = np.asarray(inputs["wg2"], np.float32)
    dwg = (wg2[:, 0] - wg2[:, 1]).reshape(Dh, 1)
    dwgh_, dwgl_ = _bf16_split(dwg)
    b20 = np.asarray(inputs["fc2_b0"], np.float32)
    b21_ = np.asarray(inputs["fc2_b1"], np.float32)

    shared = {
        "w0h": w0Th, "w0l": w0Tl, "w1h": w1Th,
        "bbh": bbh_, "bbl": bbl_, "kw": kw_, "dwb": dwb_,
        "v0h": v0Th, "v0l": v0Tl, "v1h": v1Th,
        "dwgh": dwgh_, "dwgl": dwgl_,
        "db2": (b20 - b21_).reshape(1, C), "b21": b21_.reshape(1, C),
    }
    in_maps = []
    for c in range(NCORES):
        tsl = slice(c * TPC, (c + 1) * TPC)
        m = dict(shared)
        m["x0h"] = np.ascontiguousarray(x0Th[:, tsl])
        m["x0l"] = np.ascontiguousarray(x0Tl[:, tsl])
        m["x1h"] = np.ascontiguousarray(x1Th[:, tsl])
        m["x1l"] = np.ascontiguousarray(x1Tl[:, tsl])
        m["xm"] = np.ascontiguousarray(xmb[:, tsl])
        in_maps.append(m)
    return in_maps


def kernel(**inputs):
    from concourse.bass_utils import run_bass_kernel_spmd

    assert int(inputs["H"]) == HH and int(inputs["W"]) == WW
    bias1 = bool(np.any(np.asarray(inputs["fc1_b0"])) or
                 np.any(np.asarray(inputs["fc1_b1"])))
    bias2 = bool(np.any(np.asarray(inputs["fc2_b0"])) or
                 np.any(np.asarray(inputs["fc2_b1"])))
    key = ("nc", bias1, bias2)
    if key not in _CACHE:
        _CACHE[key] = _build(bias1=bias1, bias2=bias2)
    _CACHE["nc"] = _CACHE[key]
    nc = _CACHE[key]
    in_maps = _prep_host(inputs)
    res = run_bass_kernel_spmd(nc, in_maps, list(range(NCORES)))
    y = np.concatenate([r["y"] for r in res.results], axis=0)  # [B*N, C]
    return y.reshape(B, N, C)
